# revision 15
# baseline (speedup 1.0000x reference)
"""Trainium2 Bass kernel for nn_CrossAttention (B=4, C=128, S=32, H=128, W=512).

Strategy (8 NeuronCores, SPMD single program):
  core c -> (batch b=c//2, W-half j=c%2).
  Each core: conv_block K/V over its 256-col half of y (streamed in 8
  W-tiles of 32, fused: K/V never touch DRAM), conv_block Q over x
  (duplicated per pair), attention computed per K/V column for the
  queries whose u falls in that column (host assigns queries to
  per-column slots; Q gathered into slot order on device via one-hot
  matmul in fp16). Attention uses a transposed QK (pT[h,slot] =
  k2col.T @ Qg_slice), exp without max-subtraction (|dot| < 60), a
  fused softmax denominator (ones column appended to V^T so one matmul
  yields numerator and denominator), and col-tiled AV matmuls.
  Output: per-slot attention vectors; host scatters to the query grid.
  A second tiny kernel applies the final 3x3 proj conv+relu.

Conv matmuls run as float32r (12-bit mantissa, 1 cyc/row at
free-dim>=256); QK runs in fp16 (q/k quantized - validated 1.1e-3 rel
err on the final output); AV runs in f32r.
"""
import os
import numpy as np

import concourse.tile as tile
from concourse import bacc, mybir
from concourse.bass_utils import run_bass_kernel_spmd
from concourse.masks import make_identity

# ---- problem constants (hardcoded per spec) ----
BB, CC, S = 4, 128, 32
SS = S * S            # 1024 queries/batch
HH, WW = 128, 512
SCALE = float(32 ** -0.5)
TW = 32               # W-tile width
M = 32                # slots per column
WH = 256              # per-core W half
NSLOT = WH * M        # 8192
NTILE = WH // TW      # 8
NCORE = 8

f32 = mybir.dt.float32
f32r = mybir.dt.float32r
f16 = mybir.dt.float16
bf16 = mybir.dt.bfloat16
AF = mybir.ActivationFunctionType
ALU = mybir.AluOpType
AX = mybir.AxisListType

_NC_CACHE = {}

WNAMES = ["wq1", "wq2", "wk1", "wk2", "wv1", "wv2"]


def _conv_groups(rows, r_grp):
    """Split `rows` into groups of r_grp (last may be smaller)."""
    out = []
    r0 = 0
    while r0 < rows:
        out.append((r0, min(r_grp, rows - r0)))
        r0 += r_grp
    return out


def _chunks(lst, n):
    for i in range(0, len(lst), n):
        yield lst[i:i + n]


def build_kernel_a(reps=1):
    mdt = f32r
    nc = bacc.Bacc()

    y_tiles = nc.dram_tensor("y_tiles", [NTILE, 128, 130, 36], mdt,
                             kind="ExternalInput")
    x_slab = nc.dram_tensor("x_slab", [128, 34, 34], mdt, kind="ExternalInput")
    w_dram = {n: nc.dram_tensor(n, [128, 9, 128], mdt, kind="ExternalInput")
              for n in WNAMES}
    bias6 = nc.dram_tensor("bias6", [128, 6], f32, kind="ExternalInput")
    slotq = nc.dram_tensor("slotq", [4, 2048], f16, kind="ExternalInput")
    iota8 = nc.dram_tensor("iota8", [128, 8], f32, kind="ExternalInput")
    ones1 = nc.dram_tensor("ones1", [1, 128], f16, kind="ExternalInput")
    onesv = nc.dram_tensor("onesv", [128, 32], f32, kind="ExternalInput")
    edge = nc.dram_tensor("edge", [128, 16], f32, kind="ExternalInput")
    a_out = nc.dram_tensor("a_out", [NSLOT // 128, 128, 128], f32,
                           kind="ExternalOutput")

    from contextlib import ExitStack
    with tile.TileContext(nc) as tc, ExitStack() as ctx:
        wpool = ctx.enter_context(tc.tile_pool(name="weights", bufs=1))
        const = ctx.enter_context(tc.tile_pool(name="const", bufs=1))
        qgp = ctx.enter_context(tc.tile_pool(name="qg", bufs=1))
        ps = ctx.enter_context(tc.tile_pool(name="ps", bufs=4, space="PSUM"))
        tps = ctx.enter_context(tc.tile_pool(name="tps", bufs=2, space="PSUM"))
        aps = ctx.enter_context(tc.tile_pool(name="aps", bufs=2, space="PSUM"))

        def psum_tile(free=512):
            return ps.tile([128, free], f32, tag="ps", name="pst")

        def tp_tile(free=512, dt=f32):
            return tps.tile([128, free], dt, tag="tp", name="tpt")

        def at_tile(free=512):
            return aps.tile([128, free], f32, tag="at", name="att")

        w_sb = {}
        bias_sb = const.tile([128, 6], f32, tag="bias")
        nc.sync.dma_start(out=bias_sb[:], in_=bias6[:])
        iota_sb = const.tile([128, 8], f32, tag="iota")
        nc.sync.dma_start(out=iota_sb[:], in_=iota8[:])
        ones_sb = const.tile([1, 128], f16, tag="ones")
        nc.sync.dma_start(out=ones_sb[:], in_=ones1[:])
        onesv_sb = const.tile([128, 32], f32, tag="onesv")
        nc.sync.dma_start(out=onesv_sb[:], in_=onesv[:])
        edge_sb = const.tile([128, 16], f32, tag="edge")
        nc.sync.dma_start(out=edge_sb[:], in_=edge[:])
        sq_all = const.tile([1, 8192], f16, tag="sq")
        nc.sync.dma_start(out=sq_all[:],
                          in_=slotq[:].rearrange("a b -> () (a b)"))
        ident = const.tile([128, 128], f32, tag="ident")
        make_identity(nc, ident[:])
        ident_b = const.tile([128, 128], bf16, tag="identb")
        make_identity(nc, ident_b[:])
        zeros34 = const.tile([128, 34], f32, tag="zeros34")
        nc.vector.memset(zeros34[:], 0.0)

        def zfill(ap):
            # zero-fill an f32r view via copy (memset can't emit f32r);
            # gpsimd - the idle engine - so the DVE queue stays shallow
            nc.gpsimd.tensor_copy(out=ap, in_=zeros34[:, :ap.free_size()])

        Qg = qgp.tile([128, NSLOT], f16, tag="Qg")
        yp = ctx.enter_context(tc.tile_pool(name="ytile", bufs=3))
        y_sb = {}

        # ---------------- Q path + slot gather ----------------
        with tc.tile_pool(name="qw", bufs=1) as qwp, \
             tc.tile_pool(name="qtmp", bufs=1) as qtmp, \
             tc.tile_pool(name="qt", bufs=1) as qtp, \
             tc.tile_pool(name="msel", bufs=8) as mselp, \
             tc.tile_pool(name="bcast", bufs=2) as bcp:
            wq = {}
            for n in ("wq1", "wq2"):
                t = qwp.tile([128, 9, 128], mdt, tag=n)
                nc.sync.dma_start(out=t[:], in_=w_dram[n][:])
                wq[n] = t
            xq = qtmp.tile([128, 34, 34], mdt, tag="xq")
            nc.sync.dma_start(out=xq[:], in_=x_slab[:])
            q1 = qtmp.tile([128, 34, 34], mdt, tag="q1")
            q1f = q1[:].rearrange("p a b -> p (a b)")
            zfill(q1f[:, 0:34])                              # row 0
            zfill(q1f[:, 33 * 34:34 * 34])                   # row 33
            zfill(q1[:, :, 0:1].rearrange("p a b -> p (a b)"))    # col 0
            zfill(q1[:, :, 33:34].rearrange("p a b -> p (a b)"))  # col 33
            # conv1-Q: valid 32x32 -> q1[1:33, 1:33]
            for r0 in (0, 16):
                pt = psum_tile()
                for tap in range(9):
                    dy, dx = divmod(tap, 3)
                    nc.tensor.matmul(pt[:, :512], wq["wq1"][:, tap, :],
                                     xq[:, r0 + dy:r0 + dy + 16, dx:dx + 32],
                                     start=(tap == 0), stop=(tap == 8))
                nc.scalar.activation(out=q1[:, 1 + r0:1 + r0 + 16, 1:33],
                                     in_=pt[:, :512].rearrange(
                                         "p (a b) -> p a b", a=16),
                                     func=AF.Relu, bias=bias_sb[:, 0:1],
                                     scale=1.0)
            # conv2-Q -> Q (128, 1024) f32, scaled by SCALE
            q2 = qtmp.tile([128, 1024], f32, tag="q2")
            for r0 in (0, 16):
                pt = psum_tile()
                for tap in range(9):
                    dy, dx = divmod(tap, 3)
                    nc.tensor.matmul(pt[:, :512], wq["wq2"][:, tap, :],
                                     q1[:, r0 + dy:r0 + dy + 16, dx:dx + 32],
                                     start=(tap == 0), stop=(tap == 8))
                nc.scalar.activation(out=q2[:, r0 * 32:(r0 + 16) * 32],
                                     in_=pt[:, :512], func=AF.Identity,
                                     bias=bias_sb[:, 1:2], scale=SCALE)
            # transpose Q -> 8 chunks (q, e), fp16
            qt_all = qtp.tile([128, 8, 128], f16, tag="qt")
            for qc in range(8):
                pt = tp_tile(128)
                nc.tensor.transpose(pt[:, :128],
                                    q2[:, qc * 128:(qc + 1) * 128], ident[:])
                nc.scalar.copy(out=qt_all[:, qc, :], in_=pt[:, :128])
            # build Qg = Q gathered into slots, via one-hot matmuls (fp16)
            # (one psum bank at a time: masks precomputed per block, then
            # each 512-slot chunk accumulates over the 8 query chunks)
            for blk in range(4):
                bc = bcp.tile([128, 2048], f16, tag="bc")
                for nch in range(4):
                    pb = at_tile()
                    off = blk * 2048 + nch * 512
                    nc.tensor.matmul(pb[:, :512], ones_sb[:],
                                     sq_all[0:1, off:off + 512],
                                     start=True, stop=True)
                    nc.vector.tensor_copy(out=bc[:, nch * 512:(nch + 1) * 512],
                                          in_=pb[:, :512])
                mss = []
                for qc in range(8):
                    ms = mselp.tile([128, 2048], f16, tag="ms")
                    nc.vector.tensor_scalar(out=ms[:], in0=bc[:],
                                            scalar1=iota_sb[:, qc:qc + 1],
                                            scalar2=None, op0=ALU.is_equal)
                    mss.append(ms)
                for nch in range(4):
                    pq = at_tile()
                    for qc in range(8):
                        nc.tensor.matmul(pq[:, :512], qt_all[:, qc, :],
                                         mss[qc][:, nch * 512:(nch + 1) * 512],
                                         start=(qc == 0), stop=(qc == 7))
                    off = blk * 2048 + nch * 512
                    nc.scalar.copy(out=Qg[:, off:off + 512],
                                   in_=pq[:, :512])

        for n in ("wk1", "wk2", "wv1", "wv2"):
            t = wpool.tile([128, 9, 128], mdt, tag=n)
            nc.sync.dma_start(out=t[:], in_=w_dram[n][:])
            w_sb[n] = t
        for t0 in (0, 1):
            ytp = yp.tile([128, 130, 36], mdt, tag="yt")
            nc.sync.dma_start(out=ytp[:], in_=y_tiles[t0])
            y_sb[t0] = ytp

        # ---------------- main loop over W-tiles ----------------
        c1p = ctx.enter_context(tc.tile_pool(name="c1", bufs=2))
        k2p = ctx.enter_context(tc.tile_pool(name="k2", bufs=2))
        v2p = ctx.enter_context(tc.tile_pool(name="v2", bufs=1))
        v2tp = ctx.enter_context(tc.tile_pool(name="v2t", bufs=2))
        xpp = ctx.enter_context(tc.tile_pool(name="expp", bufs=2))
        pp = ctx.enter_context(tc.tile_pool(name="pwork", bufs=2))
        smp = ctx.enter_context(tc.tile_pool(name="smax", bufs=4))

        g1 = _conv_groups(128, 13)   # conv1 valid rows (h 0..127)
        g2 = _conv_groups(128, 16)   # conv2 rows

        tseq = list(range(NTILE)) * reps
        for ti, t in enumerate(tseq):
            if ti + 2 < len(tseq):
                ytn = yp.tile([128, 130, 36], mdt, tag="yt")
                nc.sync.dma_start(out=ytn[:], in_=y_tiles[tseq[ti + 2]])
                y_sb[ti + 2] = ytn
            yt = y_sb.pop(ti)
            k2 = v2t = None
            for (w1n, w2n, b1i, b2i, kind) in [
                    ("wk1", "wk2", 2, 3, "K"), ("wv1", "wv2", 4, 5, "V")]:
                c1 = c1p.tile([128, 130, 34], mdt, tag="c1")
                c1f = c1[:].rearrange("p a b -> p (a b)")
                zfill(c1f[:, 0:34])
                zfill(c1f[:, 129 * 34:130 * 34])
                for sg in _chunks(g1, 4):
                    pts = [psum_tile() for _ in sg]
                    for tap in range(9):
                        dy, dx = divmod(tap, 3)
                        for (r0, R), pt in zip(sg, pts):
                            nc.tensor.matmul(
                                pt[:, :R * 34], w_sb[w1n][:, tap, :],
                                yt[:, r0 + dy:r0 + dy + R, dx:dx + 34],
                                start=(tap == 0), stop=(tap == 8))
                    for gi, ((r0, R), pt) in enumerate(zip(sg, pts)):
                        if gi % 2 == 0:
                            nc.scalar.activation(
                                out=c1f[:, (1 + r0) * 34:(1 + r0 + R) * 34],
                                in_=pt[:, :R * 34], func=AF.Relu,
                                bias=bias_sb[:, b1i:b1i + 1], scale=1.0)
                        else:
                            nc.vector.tensor_scalar(
                                out=c1f[:, (1 + r0) * 34:(1 + r0 + R) * 34],
                                in0=pt[:, :R * 34],
                                scalar1=bias_sb[:, b1i:b1i + 1],
                                scalar2=0.0, op0=ALU.add, op1=ALU.max)
                # zero conv1 halo cols outside the global image (data mask).
                # Only tiles 0 / NTILE-1 can touch the image boundary (which
                # core-half it is depends on j, so the mask stays data-driven)
                if t == 0:
                    nc.gpsimd.tensor_scalar_mul(c1[:, :, 0:1], c1[:, :, 0:1],
                                                edge_sb[:, 0:1])
                elif t == NTILE - 1:
                    nc.gpsimd.tensor_scalar_mul(c1[:, :, 33:34],
                                                c1[:, :, 33:34],
                                                edge_sb[:, 2 * t + 1:2 * t + 2])
                cdt = f16 if kind == "K" else bf16
                pool2 = k2p if kind == "K" else v2p
                cv2 = pool2.tile([128, 128, 32], cdt, tag="cv2" + kind)
                cv2f = cv2[:].rearrange("p a b -> p (a b)")
                for sg in _chunks(g2, 4):
                    pts = [psum_tile() for _ in sg]
                    for tap in range(9):
                        dy, dx = divmod(tap, 3)
                        for (r0, R), pt in zip(sg, pts):
                            nc.tensor.matmul(
                                pt[:, :R * 32], w_sb[w2n][:, tap, :],
                                c1[:, r0 + dy:r0 + dy + R, dx:dx + 32],
                                start=(tap == 0), stop=(tap == 8))
                    for gi, ((r0, R), pt) in enumerate(zip(sg, pts)):
                        if gi % 2 == 0:
                            nc.vector.tensor_scalar(
                                out=cv2f[:, r0 * 32:(r0 + R) * 32],
                                in0=pt[:, :R * 32],
                                scalar1=bias_sb[:, b2i:b2i + 1],
                                scalar2=None, op0=ALU.add)
                        else:
                            nc.scalar.activation(
                                out=cv2f[:, r0 * 32:(r0 + R) * 32],
                                in_=pt[:, :R * 32], func=AF.Identity,
                                bias=bias_sb[:, b2i:b2i + 1], scale=1.0)
                if kind == "K":
                    k2 = cv2
                else:
                    # V2T: per-column PE transpose -> (h, w, e) + ones col
                    # (ones col fuses the softmax denominator into AV)
                    v2t = v2tp.tile([128, 32, 132], bf16, tag="v2t")
                    nc.gpsimd.tensor_copy(
                        out=v2t[:, :, 128:129].rearrange("p a b -> p (a b)"),
                        in_=onesv_sb[:])
                    for w0 in range(0, TW, 4):
                        ptr = tp_tile(512, bf16)
                        for wi in range(4):
                            nc.tensor.transpose(
                                ptr[:, 128 * wi:128 * (wi + 1)],
                                cv2[:, :, w0 + wi], ident_b[:])
                        nc.vector.tensor_copy(
                            out=v2t[:, w0:w0 + 4, 0:128],
                            in_=ptr[:, :512].rearrange("p (a b) -> p a b",
                                                       a=4))

            # ---------------- attention for this tile ----------------
            # QK^T: pT[h, slot] = k2col.T @ Qg slice (fp16), 16 cols/psum
            for half in range(2):
                pT = at_tile()
                for ci in range(16):
                    wl = half * 16 + ci
                    slot0 = (t * TW + wl) * M
                    nc.tensor.matmul(pT[:, 32 * ci:32 * (ci + 1)],
                                     k2[:, :, wl], Qg[:, slot0:slot0 + 32],
                                     start=True, stop=True)
                # exp (no max subtraction; |dot| < 60 validated on host)
                expP = xpp.tile([128, 512], bf16, tag="expP")
                nc.scalar.activation(out=expP[:], in_=pT[:, :512],
                                     func=AF.Exp, bias=0.0, scale=1.0)
                # AV: col-tiled, 4 cols per psum; ones col gives the
                # softmax denominator in column 128
                for g4 in range(4):
                    a_ps = at_tile(132)
                    for cg in range(4):
                        wl = half * 16 + g4 * 4 + cg
                        off = (g4 * 4 + cg) * 32
                        nc.tensor.matmul(a_ps[32 * cg:32 * (cg + 1), 0:129],
                                         expP[:, off:off + 32],
                                         v2t[:, wl, 0:129],
                                         start=True, stop=True,
                                         tile_position=(0, 32 * cg))
                    sm = smp.tile([128, 1], f32, tag="sm")
                    nc.vector.reciprocal(out=sm[:], in_=a_ps[:, 128:129])
                    a_sb = pp.tile([128, 128], f32, tag="a")
                    nc.scalar.mul(out=a_sb[:], in_=a_ps[:, 0:128],
                                  mul=sm[:, 0:1])
                    nc.sync.dma_start(out=a_out[t * 8 + half * 4 + g4],
                                       in_=a_sb[:])
    nc.compile()
    return nc


def build_kernel_b():
    mdt = f32r
    nc = bacc.Bacc()
    a_slab = nc.dram_tensor("a_slab", [128, 18, 34], mdt, kind="ExternalInput")
    wp = nc.dram_tensor("wp", [128, 9, 128], mdt, kind="ExternalInput")
    bp = nc.dram_tensor("bp", [128, 1], f32, kind="ExternalInput")
    z_out = nc.dram_tensor("z_out", [128, 512], f32, kind="ExternalOutput")

    with tile.TileContext(nc) as tc:
        with tc.tile_pool(name="sb", bufs=1) as sb, \
             tc.tile_pool(name="ps", bufs=2, space="PSUM") as ps:
            a_sb = sb.tile([128, 18, 34], mdt)
            nc.sync.dma_start(out=a_sb[:], in_=a_slab[:])
            wp_sb = sb.tile([128, 9, 128], mdt)
            nc.sync.dma_start(out=wp_sb[:], in_=wp[:])
            bp_sb = sb.tile([128, 1], f32)
            nc.sync.dma_start(out=bp_sb[:], in_=bp[:])
            pt = ps.tile([128, 512], f32)
            for tap in range(9):
                dy, dx = divmod(tap, 3)
                nc.tensor.matmul(pt[:], wp_sb[:, tap, :],
                                 a_sb[:, dy:dy + 16, dx:dx + 32],
                                 start=(tap == 0), stop=(tap == 8))
            z_sb = sb.tile([128, 512], f32)
            nc.scalar.activation(out=z_sb[:], in_=pt[:], func=AF.Relu,
                                 bias=bp_sb[:, 0:1], scale=1.0)
            nc.sync.dma_start(out=z_out[:], in_=z_sb[:])
    nc.compile()
    return nc


def _round12(a):
    b = np.ascontiguousarray(a, np.float32).view(np.uint32)
    b = (b + np.uint32(0x400)) & np.uint32(0xFFFFF800)
    return b.view(np.float32)


def _get_nc(which):
    key = which
    if key not in _NC_CACHE:
        _NC_CACHE[key] = (build_kernel_a() if which == "a"
                          else build_kernel_b())
    return _NC_CACHE[key]


def sim_exec_ns():
    """Per-core kernel time (ns) from the TimelineSim cost model, A + B.

    The axon build in this container has no NTFF profiling hook, so the
    deterministic cost-model timeline is the available hardware-time
    estimate. All 8 cores run the same SPMD program, so core 0's
    timeline is representative; the two launches are summed.
    """
    from concourse.timeline_sim import TimelineSim
    total = 0.0
    for which in ("a", "b"):
        t = TimelineSim(_get_nc(which))
        t.simulate()
        total += t.time
    return int(total)


def _prep_core_a(xr, yr, uc, wt, bias6, b, j):
    """Per-core host prep. xr/yr pre-rounded full arrays."""
    y = yr[b]                      # (128, 128, 512)
    x = xr[b]                      # (128, 32, 32)
    u = uc[b].reshape(SS)          # int64 in [0, 512)

    x_slab = np.zeros((128, 34, 34), np.float32)
    x_slab[:, 1:33, 1:33] = x

    y_slab = np.zeros((128, 130, 260), np.float32)
    lo, hi = WH * j - 2, WH * j + WH + 2
    glo, ghi = max(lo, 0), min(hi, WW)
    y_slab[:, 1:129, (glo - lo):(ghi - lo)] = y[:, :, glo:ghi]
    y_tiles = np.stack([y_slab[:, :, TW * t:TW * t + 36]
                        for t in range(NTILE)])

    local = u - WH * j
    mask = (local >= 0) & (local < WH)
    slotq = np.full((NSLOT,), 2000.0, np.float16)
    counts = np.zeros(WH, np.int64)
    for q in range(SS):
        if mask[q]:
            w = int(local[q])
            r = counts[w]
            assert r < M, f"column {w} overflows {M} slots"
            slotq[w * M + r] = float(q)
            counts[w] += 1

    edge = np.ones((128, 16), np.float32)
    if j == 0:
        edge[:, 0] = 0.0        # tile 0, col0 -> global col -1
    else:
        edge[:, 2 * (NTILE - 1) + 1] = 0.0   # last tile col33 -> global 512

    iota8 = (np.arange(8, dtype=np.float32)[None, :] * 128
             + np.arange(128, dtype=np.float32)[:, None])

    in_map = {
        "y_tiles": y_tiles,
        "x_slab": x_slab,
        "bias6": bias6,
        "slotq": slotq.reshape(4, 2048),
        "iota8": iota8,
        "ones1": np.ones((1, 128), np.float16),
        "onesv": np.ones((128, 32), np.float32),
        "edge": edge,
    }
    in_map.update(wt)
    return in_map, slotq


def kernel(x, y, u, q_w1, q_b1, q_w2, q_b2, k_w1, k_b1, k_w2, k_b2,
           v_w1, v_b1, v_w2, v_b2, proj_w, proj_b):
    x = np.asarray(x, np.float32)
    y = np.asarray(y, np.float32)
    u_in = np.asarray(u)
    uc = np.clip(u_in, 0, WW - 1).astype(np.int64)

    xr, yr = _round12(x), _round12(y)
    wsrc = {"wq1": q_w1, "wq2": q_w2, "wk1": k_w1, "wk2": k_w2,
            "wv1": v_w1, "wv2": v_w2}
    wt = {n: _round12(np.asarray(w, np.float32)
                      .transpose(1, 2, 3, 0).reshape(128, 9, 128))
          for n, w in wsrc.items()}
    bias6 = np.stack([
        np.asarray(q_b1, np.float32),
        np.asarray(q_b2, np.float32) * np.float32(SCALE),
        np.asarray(k_b1, np.float32), np.asarray(k_b2, np.float32),
        np.asarray(v_b1, np.float32), np.asarray(v_b2, np.float32),
    ], axis=1)                     # (128, 6)

    in_maps, slot_maps = [], []
    for c in range(NCORE):
        im, sq = _prep_core_a(xr, yr, uc, wt, bias6, c // 2, c % 2)
        in_maps.append(im)
        slot_maps.append(sq)

    prof = bool(int(os.environ.get("KPROF", "0")))
    kw_a = dict(trace=True, tmpdir="/tmp/kprof_a") if prof else {}
    kw_b = dict(trace=True, tmpdir="/tmp/kprof_b") if prof else {}
    if prof:
        os.makedirs("/tmp/kprof_a", exist_ok=True)
        os.makedirs("/tmp/kprof_b", exist_ok=True)

    nc_a = _get_nc("a")
    res_a = run_bass_kernel_spmd(nc_a, in_maps, list(range(NCORE)), **kw_a)

    a_full = np.zeros((BB, SS, 128), np.float32)
    for c in range(NCORE):
        flat = res_a.results[c]["a_out"].reshape(NSLOT, 128)
        sq = slot_maps[c].astype(np.float32)
        valid = sq < 1024
        a_full[c // 2][sq[valid].astype(np.int64)] = flat[valid]
    a_img = a_full.transpose(0, 2, 1).reshape(BB, 128, S, S)

    wpr = _round12(np.asarray(proj_w, np.float32)
                   .transpose(1, 2, 3, 0).reshape(128, 9, 128))
    bpr = np.asarray(proj_b, np.float32).reshape(128, 1)
    in_maps_b = []
    for c in range(NCORE):
        b, rh = c // 2, c % 2
        a_slab = np.zeros((128, 18, 34), np.float32)
        r0 = 16 * rh
        rlo, rhi = max(r0 - 1, 0), min(r0 + 17, S)
        a_slab[:, (rlo - (r0 - 1)):(rhi - (r0 - 1)), 1:33] = \
            _round12(a_img[b, :, rlo:rhi, :])
        in_maps_b.append({"a_slab": a_slab, "wp": wpr, "bp": bpr})

    nc_b = _get_nc("b")
    res_b = run_bass_kernel_spmd(nc_b, in_maps_b, list(range(NCORE)), **kw_b)
    if prof:
        global LAST_EXEC_NS, LAST_EXEC_A_NS, LAST_EXEC_B_NS
        LAST_EXEC_A_NS = res_a.exec_time_ns
        LAST_EXEC_B_NS = res_b.exec_time_ns
        if res_a.exec_time_ns is not None and res_b.exec_time_ns is not None:
            LAST_EXEC_NS = res_a.exec_time_ns + res_b.exec_time_ns

    z = np.zeros((BB, 128, S, S), np.float32)
    for c in range(NCORE):
        b, rh = c // 2, c % 2
        z[b, :, 16 * rh:16 * rh + 16, :] = \
            res_b.results[c]["z_out"].reshape(128, 16, 32)
    return z


# revision 16
# speedup vs baseline: 1.0217x; 1.0217x over previous
"""Trainium2 Bass kernel for nn_CrossAttention (B=4, C=128, S=32, H=128, W=512).

Strategy (8 NeuronCores, SPMD single program):
  core c -> (batch b=c//2, W-half j=c%2).
  Each core: conv_block K/V over its 256-col half of y (streamed in 8
  W-tiles of 32, fused: K/V never touch DRAM), conv_block Q over x
  (duplicated per pair), attention computed per K/V column for the
  queries whose u falls in that column (host assigns queries to
  per-column slots; Q gathered into slot order on device via one-hot
  matmul in fp16). Attention uses a transposed QK (pT[h,slot] =
  k2col.T @ Qg_slice), exp without max-subtraction (|dot| < 60), a
  fused softmax denominator (ones column appended to V^T so one matmul
  yields numerator and denominator), and col-tiled AV matmuls.
  Output: per-slot attention vectors; host scatters to the query grid.
  A second tiny kernel applies the final 3x3 proj conv+relu.

Conv matmuls run as float32r (12-bit mantissa, 1 cyc/row at
free-dim>=256); QK runs in fp16 (q/k quantized - validated 1.1e-3 rel
err on the final output); AV runs in f32r.
"""
import os
import numpy as np

import concourse.tile as tile
from concourse import bacc, mybir
from concourse.bass_utils import run_bass_kernel_spmd
from concourse.masks import make_identity

# ---- problem constants (hardcoded per spec) ----
BB, CC, S = 4, 128, 32
SS = S * S            # 1024 queries/batch
HH, WW = 128, 512
SCALE = float(32 ** -0.5)
TW = 32               # W-tile width
M = 32                # slots per column
WH = 256              # per-core W half
NSLOT = WH * M        # 8192
NTILE = WH // TW      # 8
NCORE = 8

f32 = mybir.dt.float32
f32r = mybir.dt.float32r
f16 = mybir.dt.float16
bf16 = mybir.dt.bfloat16
AF = mybir.ActivationFunctionType
ALU = mybir.AluOpType
AX = mybir.AxisListType

_NC_CACHE = {}

WNAMES = ["wq1", "wq2", "wk1", "wk2", "wv1", "wv2"]


def _conv_groups(rows, r_grp):
    """Split `rows` into groups of r_grp (last may be smaller)."""
    out = []
    r0 = 0
    while r0 < rows:
        out.append((r0, min(r_grp, rows - r0)))
        r0 += r_grp
    return out


def _chunks(lst, n):
    for i in range(0, len(lst), n):
        yield lst[i:i + n]


def build_kernel_a(reps=1):
    mdt = f32r
    nc = bacc.Bacc()

    y_tiles = nc.dram_tensor("y_tiles", [NTILE, 128, 130, 36], mdt,
                             kind="ExternalInput")
    x_slab = nc.dram_tensor("x_slab", [128, 34, 34], mdt, kind="ExternalInput")
    w_dram = {n: nc.dram_tensor(n, [128, 9, 128], mdt, kind="ExternalInput")
              for n in WNAMES}
    bias6 = nc.dram_tensor("bias6", [128, 6], f32, kind="ExternalInput")
    slotq = nc.dram_tensor("slotq", [4, 2048], f16, kind="ExternalInput")
    iota8 = nc.dram_tensor("iota8", [128, 8], f32, kind="ExternalInput")
    ones1 = nc.dram_tensor("ones1", [1, 128], f16, kind="ExternalInput")
    onesv = nc.dram_tensor("onesv", [128, 32], f32, kind="ExternalInput")
    edge = nc.dram_tensor("edge", [128, 16], f32, kind="ExternalInput")
    a_out = nc.dram_tensor("a_out", [NSLOT // 128, 128, 128], f32,
                           kind="ExternalOutput")

    from contextlib import ExitStack
    with tile.TileContext(nc) as tc, ExitStack() as ctx:
        wpool = ctx.enter_context(tc.tile_pool(name="weights", bufs=1))
        const = ctx.enter_context(tc.tile_pool(name="const", bufs=1))
        qgp = ctx.enter_context(tc.tile_pool(name="qg", bufs=1))
        ps = ctx.enter_context(tc.tile_pool(name="ps", bufs=4, space="PSUM"))
        tps = ctx.enter_context(tc.tile_pool(name="tps", bufs=2, space="PSUM"))
        aps = ctx.enter_context(tc.tile_pool(name="aps", bufs=2, space="PSUM"))

        def psum_tile(free=512):
            return ps.tile([128, free], f32, tag="ps", name="pst")

        def tp_tile(free=512, dt=f32):
            return tps.tile([128, free], dt, tag="tp", name="tpt")

        def at_tile(free=512):
            return aps.tile([128, free], f32, tag="at", name="att")

        w_sb = {}
        bias_sb = const.tile([128, 6], f32, tag="bias")
        nc.sync.dma_start(out=bias_sb[:], in_=bias6[:])
        iota_sb = const.tile([128, 8], f32, tag="iota")
        nc.sync.dma_start(out=iota_sb[:], in_=iota8[:])
        ones_sb = const.tile([1, 128], f16, tag="ones")
        nc.sync.dma_start(out=ones_sb[:], in_=ones1[:])
        onesv_sb = const.tile([128, 32], f32, tag="onesv")
        nc.sync.dma_start(out=onesv_sb[:], in_=onesv[:])
        edge_sb = const.tile([128, 16], f32, tag="edge")
        nc.sync.dma_start(out=edge_sb[:], in_=edge[:])
        sq_all = const.tile([1, 8192], f16, tag="sq")
        nc.sync.dma_start(out=sq_all[:],
                          in_=slotq[:].rearrange("a b -> () (a b)"))
        ident = const.tile([128, 128], f32, tag="ident")
        make_identity(nc, ident[:])
        ident_b = const.tile([128, 128], bf16, tag="identb")
        make_identity(nc, ident_b[:])
        zeros34 = const.tile([128, 34], f32, tag="zeros34")
        nc.vector.memset(zeros34[:], 0.0)

        def zfill(ap):
            # zero-fill an f32r view via copy (memset can't emit f32r);
            # gpsimd - the idle engine - so the DVE queue stays shallow
            nc.gpsimd.tensor_copy(out=ap, in_=zeros34[:, :ap.free_size()])

        Qg = qgp.tile([128, NSLOT], f16, tag="Qg")
        yp = ctx.enter_context(tc.tile_pool(name="ytile", bufs=3))
        y_sb = {}

        # ---------------- Q path + slot gather ----------------
        with tc.tile_pool(name="qw", bufs=1) as qwp, \
             tc.tile_pool(name="qtmp", bufs=1) as qtmp, \
             tc.tile_pool(name="qt", bufs=1) as qtp, \
             tc.tile_pool(name="msel", bufs=8) as mselp, \
             tc.tile_pool(name="bcast", bufs=2) as bcp:
            wq = {}
            for n in ("wq1", "wq2"):
                t = qwp.tile([128, 9, 128], mdt, tag=n)
                nc.sync.dma_start(out=t[:], in_=w_dram[n][:])
                wq[n] = t
            xq = qtmp.tile([128, 34, 34], mdt, tag="xq")
            nc.sync.dma_start(out=xq[:], in_=x_slab[:])
            q1 = qtmp.tile([128, 34, 34], mdt, tag="q1")
            q1f = q1[:].rearrange("p a b -> p (a b)")
            zfill(q1f[:, 0:34])                              # row 0
            zfill(q1f[:, 33 * 34:34 * 34])                   # row 33
            zfill(q1[:, :, 0:1].rearrange("p a b -> p (a b)"))    # col 0
            zfill(q1[:, :, 33:34].rearrange("p a b -> p (a b)"))  # col 33
            # conv1-Q: valid 32x32 -> q1[1:33, 1:33]
            for r0 in (0, 16):
                pt = psum_tile()
                for tap in range(9):
                    dy, dx = divmod(tap, 3)
                    nc.tensor.matmul(pt[:, :512], wq["wq1"][:, tap, :],
                                     xq[:, r0 + dy:r0 + dy + 16, dx:dx + 32],
                                     start=(tap == 0), stop=(tap == 8))
                nc.scalar.activation(out=q1[:, 1 + r0:1 + r0 + 16, 1:33],
                                     in_=pt[:, :512].rearrange(
                                         "p (a b) -> p a b", a=16),
                                     func=AF.Relu, bias=bias_sb[:, 0:1],
                                     scale=1.0)
            # conv2-Q -> Q (128, 1024) f32, scaled by SCALE
            q2 = qtmp.tile([128, 1024], f32, tag="q2")
            for r0 in (0, 16):
                pt = psum_tile()
                for tap in range(9):
                    dy, dx = divmod(tap, 3)
                    nc.tensor.matmul(pt[:, :512], wq["wq2"][:, tap, :],
                                     q1[:, r0 + dy:r0 + dy + 16, dx:dx + 32],
                                     start=(tap == 0), stop=(tap == 8))
                nc.scalar.activation(out=q2[:, r0 * 32:(r0 + 16) * 32],
                                     in_=pt[:, :512], func=AF.Identity,
                                     bias=bias_sb[:, 1:2], scale=SCALE)
            # transpose Q -> 8 chunks (q, e), fp16
            qt_all = qtp.tile([128, 8, 128], f16, tag="qt")
            for qc in range(8):
                pt = tp_tile(128)
                nc.tensor.transpose(pt[:, :128],
                                    q2[:, qc * 128:(qc + 1) * 128], ident[:])
                nc.scalar.copy(out=qt_all[:, qc, :], in_=pt[:, :128])
            # build Qg = Q gathered into slots, via one-hot matmuls (fp16)
            # (one psum bank at a time: masks precomputed per block, then
            # each 512-slot chunk accumulates over the 8 query chunks)
            for blk in range(4):
                bc = bcp.tile([128, 2048], f16, tag="bc")
                for nch in range(4):
                    pb = at_tile()
                    off = blk * 2048 + nch * 512
                    nc.tensor.matmul(pb[:, :512], ones_sb[:],
                                     sq_all[0:1, off:off + 512],
                                     start=True, stop=True)
                    nc.vector.tensor_copy(out=bc[:, nch * 512:(nch + 1) * 512],
                                          in_=pb[:, :512])
                mss = []
                for qc in range(8):
                    ms = mselp.tile([128, 2048], f16, tag="ms")
                    nc.vector.tensor_scalar(out=ms[:], in0=bc[:],
                                            scalar1=iota_sb[:, qc:qc + 1],
                                            scalar2=None, op0=ALU.is_equal)
                    mss.append(ms)
                for nch in range(4):
                    pq = at_tile()
                    for qc in range(8):
                        nc.tensor.matmul(pq[:, :512], qt_all[:, qc, :],
                                         mss[qc][:, nch * 512:(nch + 1) * 512],
                                         start=(qc == 0), stop=(qc == 7))
                    off = blk * 2048 + nch * 512
                    nc.scalar.copy(out=Qg[:, off:off + 512],
                                   in_=pq[:, :512])

        for n in ("wk1", "wk2", "wv1", "wv2"):
            t = wpool.tile([128, 9, 128], mdt, tag=n)
            nc.sync.dma_start(out=t[:], in_=w_dram[n][:])
            w_sb[n] = t
        for t0 in (0, 1):
            ytp = yp.tile([128, 130, 36], mdt, tag="yt")
            nc.sync.dma_start(out=ytp[:], in_=y_tiles[t0])
            y_sb[t0] = ytp

        # ---------------- main loop over W-tiles ----------------
        c1p = ctx.enter_context(tc.tile_pool(name="c1", bufs=3))
        k2p = ctx.enter_context(tc.tile_pool(name="k2", bufs=2))
        v2p = ctx.enter_context(tc.tile_pool(name="v2", bufs=1))
        v2tp = ctx.enter_context(tc.tile_pool(name="v2t", bufs=2))
        xpp = ctx.enter_context(tc.tile_pool(name="expp", bufs=2))
        pp = ctx.enter_context(tc.tile_pool(name="pwork", bufs=2))
        smp = ctx.enter_context(tc.tile_pool(name="smax", bufs=4))

        g1 = _conv_groups(128, 13)   # conv1 valid rows (h 0..127)
        g2 = _conv_groups(128, 16)   # conv2 rows

        prev_c1 = {"K": None, "V": None}
        tseq = list(range(NTILE)) * reps
        for ti, t in enumerate(tseq):
            if ti + 2 < len(tseq):
                ytn = yp.tile([128, 130, 36], mdt, tag="yt")
                nc.sync.dma_start(out=ytn[:], in_=y_tiles[tseq[ti + 2]])
                y_sb[ti + 2] = ytn
            yt = y_sb.pop(ti)
            k2 = v2t = None
            for (w1n, w2n, b1i, b2i, kind) in [
                    ("wk1", "wk2", 2, 3, "K"), ("wv1", "wv2", 4, 5, "V")]:
                c1 = c1p.tile([128, 130, 34], mdt, tag="c1")
                c1f = c1[:].rearrange("p a b -> p (a b)")
                zfill(c1f[:, 0:34])
                zfill(c1f[:, 129 * 34:130 * 34])
                # halo reuse: cols 0,1 of this tile = cols 32,33 of the
                # previous tile's conv1 output (same global columns); only
                # tile 0 computes the full 34-col window
                fresh = t == 0
                c0 = 0 if fresh else 2
                CW = 34 - c0
                if not fresh:
                    nc.gpsimd.tensor_copy(out=c1[:, :, 0:2],
                                          in_=prev_c1[kind][:, :, 32:34])
                prev_c1[kind] = c1
                for sg in _chunks(g1, 4):
                    pts = [psum_tile() for _ in sg]
                    for tap in range(9):
                        dy, dx = divmod(tap, 3)
                        for (r0, R), pt in zip(sg, pts):
                            nc.tensor.matmul(
                                pt[:, :R * CW], w_sb[w1n][:, tap, :],
                                yt[:, r0 + dy:r0 + dy + R,
                                   dx + c0:dx + 34],
                                start=(tap == 0), stop=(tap == 8))
                    for gi, ((r0, R), pt) in enumerate(zip(sg, pts)):
                        pin = pt[:, :R * CW].rearrange("p (a b) -> p a b",
                                                       a=R)
                        if gi % 2 == 0:
                            nc.scalar.activation(
                                out=c1[:, 1 + r0:1 + r0 + R, c0:34],
                                in_=pin, func=AF.Relu,
                                bias=bias_sb[:, b1i:b1i + 1], scale=1.0)
                        else:
                            nc.vector.tensor_scalar(
                                out=c1[:, 1 + r0:1 + r0 + R, c0:34],
                                in0=pin,
                                scalar1=bias_sb[:, b1i:b1i + 1],
                                scalar2=0.0, op0=ALU.add, op1=ALU.max)
                # zero conv1 halo cols outside the global image (data mask).
                # Only tiles 0 / NTILE-1 can touch the image boundary (which
                # core-half it is depends on j, so the mask stays data-driven)
                if t == 0:
                    nc.gpsimd.tensor_scalar_mul(c1[:, :, 0:1], c1[:, :, 0:1],
                                                edge_sb[:, 0:1])
                elif t == NTILE - 1:
                    nc.gpsimd.tensor_scalar_mul(c1[:, :, 33:34],
                                                c1[:, :, 33:34],
                                                edge_sb[:, 2 * t + 1:2 * t + 2])
                cdt = f16 if kind == "K" else bf16
                pool2 = k2p if kind == "K" else v2p
                cv2 = pool2.tile([128, 128, 32], cdt, tag="cv2" + kind)
                cv2f = cv2[:].rearrange("p a b -> p (a b)")
                for sg in _chunks(g2, 4):
                    pts = [psum_tile() for _ in sg]
                    for tap in range(9):
                        dy, dx = divmod(tap, 3)
                        for (r0, R), pt in zip(sg, pts):
                            nc.tensor.matmul(
                                pt[:, :R * 32], w_sb[w2n][:, tap, :],
                                c1[:, r0 + dy:r0 + dy + R, dx:dx + 32],
                                start=(tap == 0), stop=(tap == 8))
                    for gi, ((r0, R), pt) in enumerate(zip(sg, pts)):
                        if gi % 2 == 0:
                            nc.vector.tensor_scalar(
                                out=cv2f[:, r0 * 32:(r0 + R) * 32],
                                in0=pt[:, :R * 32],
                                scalar1=bias_sb[:, b2i:b2i + 1],
                                scalar2=None, op0=ALU.add)
                        else:
                            nc.scalar.activation(
                                out=cv2f[:, r0 * 32:(r0 + R) * 32],
                                in_=pt[:, :R * 32], func=AF.Identity,
                                bias=bias_sb[:, b2i:b2i + 1], scale=1.0)
                if kind == "K":
                    k2 = cv2
                else:
                    # V2T: per-column PE transpose -> (h, w, e) + ones col
                    # (ones col fuses the softmax denominator into AV)
                    v2t = v2tp.tile([128, 32, 132], bf16, tag="v2t")
                    nc.gpsimd.tensor_copy(
                        out=v2t[:, :, 128:129].rearrange("p a b -> p (a b)"),
                        in_=onesv_sb[:])
                    for w0 in range(0, TW, 4):
                        ptr = tp_tile(512, bf16)
                        for wi in range(4):
                            nc.tensor.transpose(
                                ptr[:, 128 * wi:128 * (wi + 1)],
                                cv2[:, :, w0 + wi], ident_b[:])
                        nc.vector.tensor_copy(
                            out=v2t[:, w0:w0 + 4, 0:128],
                            in_=ptr[:, :512].rearrange("p (a b) -> p a b",
                                                       a=4))

            # ---------------- attention for this tile ----------------
            # QK^T: pT[h, slot] = k2col.T @ Qg slice (fp16), 16 cols/psum
            for half in range(2):
                pT = at_tile()
                for ci in range(16):
                    wl = half * 16 + ci
                    slot0 = (t * TW + wl) * M
                    nc.tensor.matmul(pT[:, 32 * ci:32 * (ci + 1)],
                                     k2[:, :, wl], Qg[:, slot0:slot0 + 32],
                                     start=True, stop=True)
                # exp (no max subtraction; |dot| < 60 validated on host)
                expP = xpp.tile([128, 512], bf16, tag="expP")
                nc.scalar.activation(out=expP[:], in_=pT[:, :512],
                                     func=AF.Exp, bias=0.0, scale=1.0)
                # AV: col-tiled, 4 cols per psum; ones col gives the
                # softmax denominator in column 128
                for g4 in range(4):
                    a_ps = at_tile(132)
                    for cg in range(4):
                        wl = half * 16 + g4 * 4 + cg
                        off = (g4 * 4 + cg) * 32
                        nc.tensor.matmul(a_ps[32 * cg:32 * (cg + 1), 0:129],
                                         expP[:, off:off + 32],
                                         v2t[:, wl, 0:129],
                                         start=True, stop=True,
                                         tile_position=(0, 32 * cg))
                    sm = smp.tile([128, 1], f32, tag="sm")
                    nc.vector.reciprocal(out=sm[:], in_=a_ps[:, 128:129])
                    a_sb = pp.tile([128, 128], f32, tag="a")
                    nc.scalar.mul(out=a_sb[:], in_=a_ps[:, 0:128],
                                  mul=sm[:, 0:1])
                    nc.sync.dma_start(out=a_out[t * 8 + half * 4 + g4],
                                       in_=a_sb[:])
    nc.compile()
    return nc


def build_kernel_b():
    mdt = f32r
    nc = bacc.Bacc()
    a_slab = nc.dram_tensor("a_slab", [128, 18, 34], mdt, kind="ExternalInput")
    wp = nc.dram_tensor("wp", [128, 9, 128], mdt, kind="ExternalInput")
    bp = nc.dram_tensor("bp", [128, 1], f32, kind="ExternalInput")
    z_out = nc.dram_tensor("z_out", [128, 512], f32, kind="ExternalOutput")

    with tile.TileContext(nc) as tc:
        with tc.tile_pool(name="sb", bufs=1) as sb, \
             tc.tile_pool(name="ps", bufs=2, space="PSUM") as ps:
            a_sb = sb.tile([128, 18, 34], mdt)
            nc.sync.dma_start(out=a_sb[:], in_=a_slab[:])
            wp_sb = sb.tile([128, 9, 128], mdt)
            nc.sync.dma_start(out=wp_sb[:], in_=wp[:])
            bp_sb = sb.tile([128, 1], f32)
            nc.sync.dma_start(out=bp_sb[:], in_=bp[:])
            pt = ps.tile([128, 512], f32)
            for tap in range(9):
                dy, dx = divmod(tap, 3)
                nc.tensor.matmul(pt[:], wp_sb[:, tap, :],
                                 a_sb[:, dy:dy + 16, dx:dx + 32],
                                 start=(tap == 0), stop=(tap == 8))
            z_sb = sb.tile([128, 512], f32)
            nc.scalar.activation(out=z_sb[:], in_=pt[:], func=AF.Relu,
                                 bias=bp_sb[:, 0:1], scale=1.0)
            nc.sync.dma_start(out=z_out[:], in_=z_sb[:])
    nc.compile()
    return nc


def _round12(a):
    b = np.ascontiguousarray(a, np.float32).view(np.uint32)
    b = (b + np.uint32(0x400)) & np.uint32(0xFFFFF800)
    return b.view(np.float32)


def _get_nc(which):
    key = which
    if key not in _NC_CACHE:
        _NC_CACHE[key] = (build_kernel_a() if which == "a"
                          else build_kernel_b())
    return _NC_CACHE[key]


def sim_exec_ns():
    """Per-core kernel time (ns) from the TimelineSim cost model, A + B.

    The axon build in this container has no NTFF profiling hook, so the
    deterministic cost-model timeline is the available hardware-time
    estimate. All 8 cores run the same SPMD program, so core 0's
    timeline is representative; the two launches are summed.
    """
    from concourse.timeline_sim import TimelineSim
    total = 0.0
    for which in ("a", "b"):
        t = TimelineSim(_get_nc(which))
        t.simulate()
        total += t.time
    return int(total)


def _prep_core_a(xr, yr, uc, wt, bias6, b, j):
    """Per-core host prep. xr/yr pre-rounded full arrays."""
    y = yr[b]                      # (128, 128, 512)
    x = xr[b]                      # (128, 32, 32)
    u = uc[b].reshape(SS)          # int64 in [0, 512)

    x_slab = np.zeros((128, 34, 34), np.float32)
    x_slab[:, 1:33, 1:33] = x

    y_slab = np.zeros((128, 130, 260), np.float32)
    lo, hi = WH * j - 2, WH * j + WH + 2
    glo, ghi = max(lo, 0), min(hi, WW)
    y_slab[:, 1:129, (glo - lo):(ghi - lo)] = y[:, :, glo:ghi]
    y_tiles = np.stack([y_slab[:, :, TW * t:TW * t + 36]
                        for t in range(NTILE)])

    local = u - WH * j
    mask = (local >= 0) & (local < WH)
    slotq = np.full((NSLOT,), 2000.0, np.float16)
    counts = np.zeros(WH, np.int64)
    for q in range(SS):
        if mask[q]:
            w = int(local[q])
            r = counts[w]
            assert r < M, f"column {w} overflows {M} slots"
            slotq[w * M + r] = float(q)
            counts[w] += 1

    edge = np.ones((128, 16), np.float32)
    if j == 0:
        edge[:, 0] = 0.0        # tile 0, col0 -> global col -1
    else:
        edge[:, 2 * (NTILE - 1) + 1] = 0.0   # last tile col33 -> global 512

    iota8 = (np.arange(8, dtype=np.float32)[None, :] * 128
             + np.arange(128, dtype=np.float32)[:, None])

    in_map = {
        "y_tiles": y_tiles,
        "x_slab": x_slab,
        "bias6": bias6,
        "slotq": slotq.reshape(4, 2048),
        "iota8": iota8,
        "ones1": np.ones((1, 128), np.float16),
        "onesv": np.ones((128, 32), np.float32),
        "edge": edge,
    }
    in_map.update(wt)
    return in_map, slotq


def kernel(x, y, u, q_w1, q_b1, q_w2, q_b2, k_w1, k_b1, k_w2, k_b2,
           v_w1, v_b1, v_w2, v_b2, proj_w, proj_b):
    x = np.asarray(x, np.float32)
    y = np.asarray(y, np.float32)
    u_in = np.asarray(u)
    uc = np.clip(u_in, 0, WW - 1).astype(np.int64)

    xr, yr = _round12(x), _round12(y)
    wsrc = {"wq1": q_w1, "wq2": q_w2, "wk1": k_w1, "wk2": k_w2,
            "wv1": v_w1, "wv2": v_w2}
    wt = {n: _round12(np.asarray(w, np.float32)
                      .transpose(1, 2, 3, 0).reshape(128, 9, 128))
          for n, w in wsrc.items()}
    bias6 = np.stack([
        np.asarray(q_b1, np.float32),
        np.asarray(q_b2, np.float32) * np.float32(SCALE),
        np.asarray(k_b1, np.float32), np.asarray(k_b2, np.float32),
        np.asarray(v_b1, np.float32), np.asarray(v_b2, np.float32),
    ], axis=1)                     # (128, 6)

    in_maps, slot_maps = [], []
    for c in range(NCORE):
        im, sq = _prep_core_a(xr, yr, uc, wt, bias6, c // 2, c % 2)
        in_maps.append(im)
        slot_maps.append(sq)

    prof = bool(int(os.environ.get("KPROF", "0")))
    kw_a = dict(trace=True, tmpdir="/tmp/kprof_a") if prof else {}
    kw_b = dict(trace=True, tmpdir="/tmp/kprof_b") if prof else {}
    if prof:
        os.makedirs("/tmp/kprof_a", exist_ok=True)
        os.makedirs("/tmp/kprof_b", exist_ok=True)

    nc_a = _get_nc("a")
    res_a = run_bass_kernel_spmd(nc_a, in_maps, list(range(NCORE)), **kw_a)

    a_full = np.zeros((BB, SS, 128), np.float32)
    for c in range(NCORE):
        flat = res_a.results[c]["a_out"].reshape(NSLOT, 128)
        sq = slot_maps[c].astype(np.float32)
        valid = sq < 1024
        a_full[c // 2][sq[valid].astype(np.int64)] = flat[valid]
    a_img = a_full.transpose(0, 2, 1).reshape(BB, 128, S, S)

    wpr = _round12(np.asarray(proj_w, np.float32)
                   .transpose(1, 2, 3, 0).reshape(128, 9, 128))
    bpr = np.asarray(proj_b, np.float32).reshape(128, 1)
    in_maps_b = []
    for c in range(NCORE):
        b, rh = c // 2, c % 2
        a_slab = np.zeros((128, 18, 34), np.float32)
        r0 = 16 * rh
        rlo, rhi = max(r0 - 1, 0), min(r0 + 17, S)
        a_slab[:, (rlo - (r0 - 1)):(rhi - (r0 - 1)), 1:33] = \
            _round12(a_img[b, :, rlo:rhi, :])
        in_maps_b.append({"a_slab": a_slab, "wp": wpr, "bp": bpr})

    nc_b = _get_nc("b")
    res_b = run_bass_kernel_spmd(nc_b, in_maps_b, list(range(NCORE)), **kw_b)
    if prof:
        global LAST_EXEC_NS, LAST_EXEC_A_NS, LAST_EXEC_B_NS
        LAST_EXEC_A_NS = res_a.exec_time_ns
        LAST_EXEC_B_NS = res_b.exec_time_ns
        if res_a.exec_time_ns is not None and res_b.exec_time_ns is not None:
            LAST_EXEC_NS = res_a.exec_time_ns + res_b.exec_time_ns

    z = np.zeros((BB, 128, S, S), np.float32)
    for c in range(NCORE):
        b, rh = c // 2, c % 2
        z[b, :, 16 * rh:16 * rh + 16, :] = \
            res_b.results[c]["z_out"].reshape(128, 16, 32)
    return z


# revision 18
# speedup vs baseline: 1.0219x; 1.0002x over previous
"""Trainium2 Bass kernel for nn_CrossAttention (B=4, C=128, S=32, H=128, W=512).

Strategy (8 NeuronCores, SPMD single program):
  core c -> (batch b=c//2, W-half j=c%2).
  Each core: conv_block K/V over its 256-col half of y (streamed in 8
  W-tiles of 32, fused: K/V never touch DRAM), conv_block Q over x
  (duplicated per pair), attention computed per K/V column for the
  queries whose u falls in that column (host assigns queries to
  per-column slots; Q gathered into slot order on device via one-hot
  matmul in fp16). Attention uses a transposed QK (pT[h,slot] =
  k2col.T @ Qg_slice), exp without max-subtraction (|dot| < 60), a
  fused softmax denominator (ones column appended to V^T so one matmul
  yields numerator and denominator), and col-tiled AV matmuls.
  Output: per-slot attention vectors; host scatters to the query grid.
  A second tiny kernel applies the final 3x3 proj conv+relu.

Conv matmuls run as float32r (12-bit mantissa, 1 cyc/row at
free-dim>=256); QK runs in fp16 (q/k quantized - validated 1.1e-3 rel
err on the final output); AV runs in f32r.
"""
import os
import numpy as np

import concourse.tile as tile
from concourse import bacc, mybir
from concourse.bass_utils import run_bass_kernel_spmd
from concourse.masks import make_identity

# ---- problem constants (hardcoded per spec) ----
BB, CC, S = 4, 128, 32
SS = S * S            # 1024 queries/batch
HH, WW = 128, 512
SCALE = float(32 ** -0.5)
TW = 32               # W-tile width
M = 32                # slots per column
WH = 256              # per-core W half
NSLOT = WH * M        # 8192
NTILE = WH // TW      # 8
NCORE = 8

f32 = mybir.dt.float32
f32r = mybir.dt.float32r
f16 = mybir.dt.float16
bf16 = mybir.dt.bfloat16
AF = mybir.ActivationFunctionType
ALU = mybir.AluOpType
AX = mybir.AxisListType

_NC_CACHE = {}

WNAMES = ["wq1", "wq2", "wk1", "wk2", "wv1", "wv2"]


def _conv_groups(rows, r_grp):
    """Split `rows` into groups of r_grp (last may be smaller)."""
    out = []
    r0 = 0
    while r0 < rows:
        out.append((r0, min(r_grp, rows - r0)))
        r0 += r_grp
    return out


def _chunks(lst, n):
    for i in range(0, len(lst), n):
        yield lst[i:i + n]


def build_kernel_a(reps=1):
    mdt = f32r
    nc = bacc.Bacc()

    y_tiles = nc.dram_tensor("y_tiles", [NTILE, 128, 130, 36], mdt,
                             kind="ExternalInput")
    x_slab = nc.dram_tensor("x_slab", [128, 34, 34], mdt, kind="ExternalInput")
    w_dram = {n: nc.dram_tensor(n, [128, 9, 128], mdt, kind="ExternalInput")
              for n in WNAMES}
    bias6 = nc.dram_tensor("bias6", [128, 6], f32, kind="ExternalInput")
    slotq = nc.dram_tensor("slotq", [4, 2048], f16, kind="ExternalInput")
    iota8 = nc.dram_tensor("iota8", [128, 8], f32, kind="ExternalInput")
    ones1 = nc.dram_tensor("ones1", [1, 128], f16, kind="ExternalInput")
    onesv = nc.dram_tensor("onesv", [128, 32], f32, kind="ExternalInput")
    edge = nc.dram_tensor("edge", [128, 16], f32, kind="ExternalInput")
    a_out = nc.dram_tensor("a_out", [NSLOT // 128, 128, 128], f32,
                           kind="ExternalOutput")

    from contextlib import ExitStack
    with tile.TileContext(nc) as tc, ExitStack() as ctx:
        wpool = ctx.enter_context(tc.tile_pool(name="weights", bufs=1))
        const = ctx.enter_context(tc.tile_pool(name="const", bufs=1))
        qgp = ctx.enter_context(tc.tile_pool(name="qg", bufs=1))
        ps = ctx.enter_context(tc.tile_pool(name="ps", bufs=4, space="PSUM"))
        tps = ctx.enter_context(tc.tile_pool(name="tps", bufs=2, space="PSUM"))
        aps = ctx.enter_context(tc.tile_pool(name="aps", bufs=2, space="PSUM"))

        def psum_tile(free=512):
            return ps.tile([128, free], f32, tag="ps", name="pst")

        def tp_tile(free=512, dt=f32):
            return tps.tile([128, free], dt, tag="tp", name="tpt")

        def at_tile(free=512):
            return aps.tile([128, free], f32, tag="at", name="att")

        w_sb = {}
        bias_sb = const.tile([128, 6], f32, tag="bias")
        nc.sync.dma_start(out=bias_sb[:], in_=bias6[:])
        iota_sb = const.tile([128, 8], f32, tag="iota")
        nc.sync.dma_start(out=iota_sb[:], in_=iota8[:])
        ones_sb = const.tile([1, 128], f16, tag="ones")
        nc.sync.dma_start(out=ones_sb[:], in_=ones1[:])
        onesv_sb = const.tile([128, 32], f32, tag="onesv")
        nc.sync.dma_start(out=onesv_sb[:], in_=onesv[:])
        edge_sb = const.tile([128, 16], f32, tag="edge")
        nc.sync.dma_start(out=edge_sb[:], in_=edge[:])
        sq_all = const.tile([1, 8192], f16, tag="sq")
        nc.sync.dma_start(out=sq_all[:],
                          in_=slotq[:].rearrange("a b -> () (a b)"))
        ident = const.tile([128, 128], f32, tag="ident")
        make_identity(nc, ident[:])
        ident_b = const.tile([128, 128], bf16, tag="identb")
        make_identity(nc, ident_b[:])
        zeros34 = const.tile([128, 34], f32, tag="zeros34")
        nc.vector.memset(zeros34[:], 0.0)

        def zfill(ap):
            # zero-fill an f32r view via copy (memset can't emit f32r);
            # gpsimd - the idle engine - so the DVE queue stays shallow
            nc.gpsimd.tensor_copy(out=ap, in_=zeros34[:, :ap.free_size()])

        Qg = qgp.tile([128, NSLOT], f16, tag="Qg")
        yp = ctx.enter_context(tc.tile_pool(name="ytile", bufs=3))
        y_sb = {}

        # ---------------- Q path + slot gather ----------------
        with tc.tile_pool(name="qw", bufs=1) as qwp, \
             tc.tile_pool(name="qtmp", bufs=1) as qtmp, \
             tc.tile_pool(name="qt", bufs=1) as qtp, \
             tc.tile_pool(name="msel", bufs=8) as mselp, \
             tc.tile_pool(name="bcast", bufs=2) as bcp:
            wq = {}
            wq["wq1"] = qwp.tile([128, 9, 128], mdt, tag="wq1", name="wq1t")
            nc.sync.dma_start(out=wq["wq1"][:], in_=w_dram["wq1"][:])
            xq = qtmp.tile([128, 34, 34], mdt, tag="xq")
            nc.sync.dma_start(out=xq[:], in_=x_slab[:])
            wq["wq2"] = qwp.tile([128, 9, 128], mdt, tag="wq2", name="wq2t")
            nc.sync.dma_start(out=wq["wq2"][:], in_=w_dram["wq2"][:])
            q1 = qtmp.tile([128, 34, 34], mdt, tag="q1")
            q1f = q1[:].rearrange("p a b -> p (a b)")
            zfill(q1f[:, 0:34])                              # row 0
            zfill(q1f[:, 33 * 34:34 * 34])                   # row 33
            zfill(q1[:, :, 0:1].rearrange("p a b -> p (a b)"))    # col 0
            zfill(q1[:, :, 33:34].rearrange("p a b -> p (a b)"))  # col 33
            # conv1-Q: valid 32x32 -> q1[1:33, 1:33]
            for r0 in (0, 16):
                pt = psum_tile()
                for tap in range(9):
                    dy, dx = divmod(tap, 3)
                    nc.tensor.matmul(pt[:, :512], wq["wq1"][:, tap, :],
                                     xq[:, r0 + dy:r0 + dy + 16, dx:dx + 32],
                                     start=(tap == 0), stop=(tap == 8))
                nc.scalar.activation(out=q1[:, 1 + r0:1 + r0 + 16, 1:33],
                                     in_=pt[:, :512].rearrange(
                                         "p (a b) -> p a b", a=16),
                                     func=AF.Relu, bias=bias_sb[:, 0:1],
                                     scale=1.0)
            # conv2-Q -> Q (128, 1024) f32, scaled by SCALE
            q2 = qtmp.tile([128, 1024], f32, tag="q2")
            for r0 in (0, 16):
                pt = psum_tile()
                for tap in range(9):
                    dy, dx = divmod(tap, 3)
                    nc.tensor.matmul(pt[:, :512], wq["wq2"][:, tap, :],
                                     q1[:, r0 + dy:r0 + dy + 16, dx:dx + 32],
                                     start=(tap == 0), stop=(tap == 8))
                nc.scalar.activation(out=q2[:, r0 * 32:(r0 + 16) * 32],
                                     in_=pt[:, :512], func=AF.Identity,
                                     bias=bias_sb[:, 1:2], scale=SCALE)
            # transpose Q -> 8 chunks (q, e), fp16
            qt_all = qtp.tile([128, 8, 128], f16, tag="qt")
            for qc in range(8):
                pt = tp_tile(128)
                nc.tensor.transpose(pt[:, :128],
                                    q2[:, qc * 128:(qc + 1) * 128], ident[:])
                nc.scalar.copy(out=qt_all[:, qc, :], in_=pt[:, :128])
            # build Qg = Q gathered into slots, via one-hot matmuls (fp16)
            # (one psum bank at a time: masks precomputed per block, then
            # each 512-slot chunk accumulates over the 8 query chunks)
            for blk in range(4):
                bc = bcp.tile([128, 2048], f16, tag="bc")
                for nch in range(4):
                    pb = at_tile()
                    off = blk * 2048 + nch * 512
                    nc.tensor.matmul(pb[:, :512], ones_sb[:],
                                     sq_all[0:1, off:off + 512],
                                     start=True, stop=True)
                    nc.vector.tensor_copy(out=bc[:, nch * 512:(nch + 1) * 512],
                                          in_=pb[:, :512])
                mss = []
                for qc in range(8):
                    ms = mselp.tile([128, 2048], f16, tag="ms")
                    nc.vector.tensor_scalar(out=ms[:], in0=bc[:],
                                            scalar1=iota_sb[:, qc:qc + 1],
                                            scalar2=None, op0=ALU.is_equal)
                    mss.append(ms)
                for nch in range(4):
                    pq = at_tile()
                    for qc in range(8):
                        nc.tensor.matmul(pq[:, :512], qt_all[:, qc, :],
                                         mss[qc][:, nch * 512:(nch + 1) * 512],
                                         start=(qc == 0), stop=(qc == 7))
                    off = blk * 2048 + nch * 512
                    nc.scalar.copy(out=Qg[:, off:off + 512],
                                   in_=pq[:, :512])

        for n in ("wk1", "wk2", "wv1", "wv2"):
            t = wpool.tile([128, 9, 128], mdt, tag=n)
            nc.sync.dma_start(out=t[:], in_=w_dram[n][:])
            w_sb[n] = t
        for t0 in (0, 1):
            ytp = yp.tile([128, 130, 36], mdt, tag="yt")
            nc.sync.dma_start(out=ytp[:], in_=y_tiles[t0])
            y_sb[t0] = ytp

        # ---------------- main loop over W-tiles ----------------
        c1p = ctx.enter_context(tc.tile_pool(name="c1", bufs=3))
        k2p = ctx.enter_context(tc.tile_pool(name="k2", bufs=2))
        v2p = ctx.enter_context(tc.tile_pool(name="v2", bufs=1))
        v2tp = ctx.enter_context(tc.tile_pool(name="v2t", bufs=2))
        xpp = ctx.enter_context(tc.tile_pool(name="expp", bufs=2))
        pp = ctx.enter_context(tc.tile_pool(name="pwork", bufs=2))
        smp = ctx.enter_context(tc.tile_pool(name="smax", bufs=4))

        g1 = _conv_groups(128, 13)   # conv1 rows, 34-col tile (R*34<=512)
        g1b = _conv_groups(128, 16)  # conv1 rows, 32-col tiles (R*32=512)
        g2 = _conv_groups(128, 16)   # conv2 rows

        prev_c1 = {"K": None, "V": None}
        tseq = list(range(NTILE)) * reps
        for ti, t in enumerate(tseq):
            if ti + 2 < len(tseq):
                ytn = yp.tile([128, 130, 36], mdt, tag="yt")
                nc.sync.dma_start(out=ytn[:], in_=y_tiles[tseq[ti + 2]])
                y_sb[ti + 2] = ytn
            yt = y_sb.pop(ti)
            k2 = v2t = None
            for (w1n, w2n, b1i, b2i, kind) in [
                    ("wk1", "wk2", 2, 3, "K"), ("wv1", "wv2", 4, 5, "V")]:
                c1 = c1p.tile([128, 130, 34], mdt, tag="c1")
                c1f = c1[:].rearrange("p a b -> p (a b)")
                zfill(c1f[:, 0:34])
                zfill(c1f[:, 129 * 34:130 * 34])
                # halo reuse: cols 0,1 of this tile = cols 32,33 of the
                # previous tile's conv1 output (same global columns); only
                # tile 0 computes the full 34-col window
                fresh = t == 0
                c0 = 0 if fresh else 2
                CW = 34 - c0
                if not fresh:
                    nc.gpsimd.tensor_copy(out=c1[:, :, 0:2],
                                          in_=prev_c1[kind][:, :, 32:34])
                prev_c1[kind] = c1
                for sg in _chunks(g1 if fresh else g1b, 4):
                    pts = [psum_tile() for _ in sg]
                    for tap in range(9):
                        dy, dx = divmod(tap, 3)
                        for (r0, R), pt in zip(sg, pts):
                            nc.tensor.matmul(
                                pt[:, :R * CW], w_sb[w1n][:, tap, :],
                                yt[:, r0 + dy:r0 + dy + R,
                                   dx + c0:dx + 34],
                                start=(tap == 0), stop=(tap == 8))
                    for gi, ((r0, R), pt) in enumerate(zip(sg, pts)):
                        pin = pt[:, :R * CW].rearrange("p (a b) -> p a b",
                                                       a=R)
                        if gi % 2 == 0:
                            nc.scalar.activation(
                                out=c1[:, 1 + r0:1 + r0 + R, c0:34],
                                in_=pin, func=AF.Relu,
                                bias=bias_sb[:, b1i:b1i + 1], scale=1.0)
                        else:
                            nc.vector.tensor_scalar(
                                out=c1[:, 1 + r0:1 + r0 + R, c0:34],
                                in0=pin,
                                scalar1=bias_sb[:, b1i:b1i + 1],
                                scalar2=0.0, op0=ALU.add, op1=ALU.max)
                # zero conv1 halo cols outside the global image (data mask).
                # Only tiles 0 / NTILE-1 can touch the image boundary (which
                # core-half it is depends on j, so the mask stays data-driven)
                if t == 0:
                    nc.gpsimd.tensor_scalar_mul(c1[:, :, 0:1], c1[:, :, 0:1],
                                                edge_sb[:, 0:1])
                elif t == NTILE - 1:
                    nc.gpsimd.tensor_scalar_mul(c1[:, :, 33:34],
                                                c1[:, :, 33:34],
                                                edge_sb[:, 2 * t + 1:2 * t + 2])
                cdt = f16 if kind == "K" else bf16
                pool2 = k2p if kind == "K" else v2p
                cv2 = pool2.tile([128, 128, 32], cdt, tag="cv2" + kind)
                cv2f = cv2[:].rearrange("p a b -> p (a b)")
                for sg in _chunks(g2, 4):
                    pts = [psum_tile() for _ in sg]
                    for tap in range(9):
                        dy, dx = divmod(tap, 3)
                        for (r0, R), pt in zip(sg, pts):
                            nc.tensor.matmul(
                                pt[:, :R * 32], w_sb[w2n][:, tap, :],
                                c1[:, r0 + dy:r0 + dy + R, dx:dx + 32],
                                start=(tap == 0), stop=(tap == 8))
                    for gi, ((r0, R), pt) in enumerate(zip(sg, pts)):
                        if gi % 2 == 0:
                            nc.vector.tensor_scalar(
                                out=cv2f[:, r0 * 32:(r0 + R) * 32],
                                in0=pt[:, :R * 32],
                                scalar1=bias_sb[:, b2i:b2i + 1],
                                scalar2=None, op0=ALU.add)
                        else:
                            nc.scalar.activation(
                                out=cv2f[:, r0 * 32:(r0 + R) * 32],
                                in_=pt[:, :R * 32], func=AF.Identity,
                                bias=bias_sb[:, b2i:b2i + 1], scale=1.0)
                if kind == "K":
                    k2 = cv2
                else:
                    # V2T: per-column PE transpose -> (h, w, e) + ones col
                    # (ones col fuses the softmax denominator into AV)
                    v2t = v2tp.tile([128, 32, 132], bf16, tag="v2t")
                    nc.gpsimd.tensor_copy(
                        out=v2t[:, :, 128:129].rearrange("p a b -> p (a b)"),
                        in_=onesv_sb[:])
                    for w0 in range(0, TW, 4):
                        ptr = tp_tile(512, bf16)
                        for wi in range(4):
                            nc.tensor.transpose(
                                ptr[:, 128 * wi:128 * (wi + 1)],
                                cv2[:, :, w0 + wi], ident_b[:])
                        nc.vector.tensor_copy(
                            out=v2t[:, w0:w0 + 4, 0:128],
                            in_=ptr[:, :512].rearrange("p (a b) -> p a b",
                                                       a=4))

            # ---------------- attention for this tile ----------------
            # QK^T: pT[h, slot] = k2col.T @ Qg slice (fp16), 16 cols/psum
            for half in range(2):
                pT = at_tile()
                for ci in range(16):
                    wl = half * 16 + ci
                    slot0 = (t * TW + wl) * M
                    nc.tensor.matmul(pT[:, 32 * ci:32 * (ci + 1)],
                                     k2[:, :, wl], Qg[:, slot0:slot0 + 32],
                                     start=True, stop=True)
                # exp (no max subtraction; |dot| < 60 validated on host)
                expP = xpp.tile([128, 512], bf16, tag="expP")
                nc.scalar.activation(out=expP[:], in_=pT[:, :512],
                                     func=AF.Exp, bias=0.0, scale=1.0)
                # AV: col-tiled, 4 cols per psum; ones col gives the
                # softmax denominator in column 128
                for g4 in range(4):
                    a_ps = at_tile(132)
                    for cg in range(4):
                        wl = half * 16 + g4 * 4 + cg
                        off = (g4 * 4 + cg) * 32
                        nc.tensor.matmul(a_ps[32 * cg:32 * (cg + 1), 0:129],
                                         expP[:, off:off + 32],
                                         v2t[:, wl, 0:129],
                                         start=True, stop=True,
                                         tile_position=(0, 32 * cg))
                    sm = smp.tile([128, 1], f32, tag="sm")
                    nc.vector.reciprocal(out=sm[:], in_=a_ps[:, 128:129])
                    a_sb = pp.tile([128, 128], f32, tag="a")
                    nc.scalar.mul(out=a_sb[:], in_=a_ps[:, 0:128],
                                  mul=sm[:, 0:1])
                    nc.sync.dma_start(out=a_out[t * 8 + half * 4 + g4],
                                       in_=a_sb[:])
    nc.compile()
    return nc


def build_kernel_b():
    mdt = f32r
    nc = bacc.Bacc()
    a_slab = nc.dram_tensor("a_slab", [128, 18, 34], mdt, kind="ExternalInput")
    wp = nc.dram_tensor("wp", [128, 9, 128], mdt, kind="ExternalInput")
    bp = nc.dram_tensor("bp", [128, 1], f32, kind="ExternalInput")
    z_out = nc.dram_tensor("z_out", [128, 512], f32, kind="ExternalOutput")

    with tile.TileContext(nc) as tc:
        with tc.tile_pool(name="sb", bufs=1) as sb, \
             tc.tile_pool(name="ps", bufs=2, space="PSUM") as ps:
            a_sb = sb.tile([128, 18, 34], mdt)
            nc.sync.dma_start(out=a_sb[:], in_=a_slab[:])
            wp_sb = sb.tile([128, 9, 128], mdt)
            nc.sync.dma_start(out=wp_sb[:], in_=wp[:])
            bp_sb = sb.tile([128, 1], f32)
            nc.sync.dma_start(out=bp_sb[:], in_=bp[:])
            pt = ps.tile([128, 512], f32)
            for tap in range(9):
                dy, dx = divmod(tap, 3)
                nc.tensor.matmul(pt[:], wp_sb[:, tap, :],
                                 a_sb[:, dy:dy + 16, dx:dx + 32],
                                 start=(tap == 0), stop=(tap == 8))
            z_sb = sb.tile([128, 512], f32)
            nc.scalar.activation(out=z_sb[:], in_=pt[:], func=AF.Relu,
                                 bias=bp_sb[:, 0:1], scale=1.0)
            nc.sync.dma_start(out=z_out[:], in_=z_sb[:])
    nc.compile()
    return nc


def _round12(a):
    b = np.ascontiguousarray(a, np.float32).view(np.uint32)
    b = (b + np.uint32(0x400)) & np.uint32(0xFFFFF800)
    return b.view(np.float32)


def _get_nc(which):
    key = which
    if key not in _NC_CACHE:
        _NC_CACHE[key] = (build_kernel_a() if which == "a"
                          else build_kernel_b())
    return _NC_CACHE[key]


def sim_exec_ns():
    """Per-core kernel time (ns) from the TimelineSim cost model, A + B.

    The axon build in this container has no NTFF profiling hook, so the
    deterministic cost-model timeline is the available hardware-time
    estimate. All 8 cores run the same SPMD program, so core 0's
    timeline is representative; the two launches are summed.
    """
    from concourse.timeline_sim import TimelineSim
    total = 0.0
    for which in ("a", "b"):
        t = TimelineSim(_get_nc(which))
        t.simulate()
        total += t.time
    return int(total)


def _prep_core_a(xr, yr, uc, wt, bias6, b, j):
    """Per-core host prep. xr/yr pre-rounded full arrays."""
    y = yr[b]                      # (128, 128, 512)
    x = xr[b]                      # (128, 32, 32)
    u = uc[b].reshape(SS)          # int64 in [0, 512)

    x_slab = np.zeros((128, 34, 34), np.float32)
    x_slab[:, 1:33, 1:33] = x

    y_slab = np.zeros((128, 130, 260), np.float32)
    lo, hi = WH * j - 2, WH * j + WH + 2
    glo, ghi = max(lo, 0), min(hi, WW)
    y_slab[:, 1:129, (glo - lo):(ghi - lo)] = y[:, :, glo:ghi]
    y_tiles = np.stack([y_slab[:, :, TW * t:TW * t + 36]
                        for t in range(NTILE)])

    local = u - WH * j
    mask = (local >= 0) & (local < WH)
    slotq = np.full((NSLOT,), 2000.0, np.float16)
    counts = np.zeros(WH, np.int64)
    for q in range(SS):
        if mask[q]:
            w = int(local[q])
            r = counts[w]
            assert r < M, f"column {w} overflows {M} slots"
            slotq[w * M + r] = float(q)
            counts[w] += 1

    edge = np.ones((128, 16), np.float32)
    if j == 0:
        edge[:, 0] = 0.0        # tile 0, col0 -> global col -1
    else:
        edge[:, 2 * (NTILE - 1) + 1] = 0.0   # last tile col33 -> global 512

    iota8 = (np.arange(8, dtype=np.float32)[None, :] * 128
             + np.arange(128, dtype=np.float32)[:, None])

    in_map = {
        "y_tiles": y_tiles,
        "x_slab": x_slab,
        "bias6": bias6,
        "slotq": slotq.reshape(4, 2048),
        "iota8": iota8,
        "ones1": np.ones((1, 128), np.float16),
        "onesv": np.ones((128, 32), np.float32),
        "edge": edge,
    }
    in_map.update(wt)
    return in_map, slotq


def kernel(x, y, u, q_w1, q_b1, q_w2, q_b2, k_w1, k_b1, k_w2, k_b2,
           v_w1, v_b1, v_w2, v_b2, proj_w, proj_b):
    x = np.asarray(x, np.float32)
    y = np.asarray(y, np.float32)
    u_in = np.asarray(u)
    uc = np.clip(u_in, 0, WW - 1).astype(np.int64)

    xr, yr = _round12(x), _round12(y)
    wsrc = {"wq1": q_w1, "wq2": q_w2, "wk1": k_w1, "wk2": k_w2,
            "wv1": v_w1, "wv2": v_w2}
    wt = {n: _round12(np.asarray(w, np.float32)
                      .transpose(1, 2, 3, 0).reshape(128, 9, 128))
          for n, w in wsrc.items()}
    bias6 = np.stack([
        np.asarray(q_b1, np.float32),
        np.asarray(q_b2, np.float32) * np.float32(SCALE),
        np.asarray(k_b1, np.float32), np.asarray(k_b2, np.float32),
        np.asarray(v_b1, np.float32), np.asarray(v_b2, np.float32),
    ], axis=1)                     # (128, 6)

    in_maps, slot_maps = [], []
    for c in range(NCORE):
        im, sq = _prep_core_a(xr, yr, uc, wt, bias6, c // 2, c % 2)
        in_maps.append(im)
        slot_maps.append(sq)

    prof = bool(int(os.environ.get("KPROF", "0")))
    kw_a = dict(trace=True, tmpdir="/tmp/kprof_a") if prof else {}
    kw_b = dict(trace=True, tmpdir="/tmp/kprof_b") if prof else {}
    if prof:
        os.makedirs("/tmp/kprof_a", exist_ok=True)
        os.makedirs("/tmp/kprof_b", exist_ok=True)

    nc_a = _get_nc("a")
    res_a = run_bass_kernel_spmd(nc_a, in_maps, list(range(NCORE)), **kw_a)

    a_full = np.zeros((BB, SS, 128), np.float32)
    for c in range(NCORE):
        flat = res_a.results[c]["a_out"].reshape(NSLOT, 128)
        sq = slot_maps[c].astype(np.float32)
        valid = sq < 1024
        a_full[c // 2][sq[valid].astype(np.int64)] = flat[valid]
    a_img = a_full.transpose(0, 2, 1).reshape(BB, 128, S, S)

    wpr = _round12(np.asarray(proj_w, np.float32)
                   .transpose(1, 2, 3, 0).reshape(128, 9, 128))
    bpr = np.asarray(proj_b, np.float32).reshape(128, 1)
    in_maps_b = []
    for c in range(NCORE):
        b, rh = c // 2, c % 2
        a_slab = np.zeros((128, 18, 34), np.float32)
        r0 = 16 * rh
        rlo, rhi = max(r0 - 1, 0), min(r0 + 17, S)
        a_slab[:, (rlo - (r0 - 1)):(rhi - (r0 - 1)), 1:33] = \
            _round12(a_img[b, :, rlo:rhi, :])
        in_maps_b.append({"a_slab": a_slab, "wp": wpr, "bp": bpr})

    nc_b = _get_nc("b")
    res_b = run_bass_kernel_spmd(nc_b, in_maps_b, list(range(NCORE)), **kw_b)
    if prof:
        global LAST_EXEC_NS, LAST_EXEC_A_NS, LAST_EXEC_B_NS
        LAST_EXEC_A_NS = res_a.exec_time_ns
        LAST_EXEC_B_NS = res_b.exec_time_ns
        if res_a.exec_time_ns is not None and res_b.exec_time_ns is not None:
            LAST_EXEC_NS = res_a.exec_time_ns + res_b.exec_time_ns

    z = np.zeros((BB, 128, S, S), np.float32)
    for c in range(NCORE):
        b, rh = c // 2, c % 2
        z[b, :, 16 * rh:16 * rh + 16, :] = \
            res_b.results[c]["z_out"].reshape(128, 16, 32)
    return z
